# revision 23
# baseline (speedup 1.0000x reference)
"""Trainium2 Bass kernel for AttnBlock: GroupNorm -> single-head attention -> out proj + residual.

Shapes: x [B=8, C=512, L=2048].  Sharding: data-parallel over batch, one batch
element per NeuronCore (8 cores), no collectives.

Per-core dataflow ([C, L] = [512, 2048]), all matmuls bf16/fp8 with fp32 PSUM:
  1. GroupNorm(32 groups of 16ch): channel/group sums computed ON THE PE via
     one-hot group matmuls (G [128,32]) that track the x DMA tile-by-tile
     (also keeps HAM warm); x^2 chunks produced on DVE/ACT and group-summed
     the same way.  Group stats broadcast back to channels via a tiny fp32
     matmul with B = G^T.  h applied per tile on DVE/ACT/GPSIMD in parallel.
  2. q, k = WT.T @ h   ([co, l] layout, fp8 DoubleRow);  vT = h.T @ WvT
     ([l, co+1] layout with a ones-channel appended).  PSUM drains alternate
     ACT/DVE so the PE never waits.  V-projection interleaved into the first
     superblock's S^T phase.
  3. Attention per 1024-wide query superblock:
       S^T[j, i] = sum_c k[c,j] q[c,i]  ->  PT = exp(scale*S^T)  (ACT, fp8)
       aT_un[i, c] = sum_j PT[j,i] vT[j,c] via two half-matmuls (256 + 257
       cols); the ones-channel of vT makes column 512 the softmax rowsum, so
       no separate reduction is needed;  aT = aT_un * (1/rowsum) on ACT;
       a[c, i] via PE transpose (pipelined one i-block behind AV).
  4. o = WoT.T @ a + bo_eff + x (bf16 x reused from the GN load - no second
     fp32 x fetch);  output projection software-pipelined behind the next
     superblock's S^T, with the last quarter-blocks interleaved into the
     final AV loop to shrink the serial tail.
"""

import os
import sys

import numpy as np

if "/opt/trn_rl_repo" not in sys.path:
    sys.path.insert(0, "/opt/trn_rl_repo")

import ml_dtypes

B, C, L = 8, 512, 2048
NG = 32  # groups
GS = C // NG  # 16 channels per group
EPS = 1e-5
P = 128  # partitions
CT = C // P  # 4 channel tiles
LT = L // P  # 16 position tiles
SCALE = 1.0 / float(np.sqrt(C))

LAST_RESULT = None  # BassKernelResults of the most recent run (for test harness)


def _build_nc():
    import concourse.bass as bass
    from concourse import bacc, mybir, tile

    dt = mybir.dt
    f32, bf16, f8 = dt.float32, dt.bfloat16, dt.float8e4
    AF = mybir.ActivationFunctionType
    OP = mybir.AluOpType

    nc = bacc.Bacc()

    xf8_d = nc.declare_dram_parameter("xf8", [C, L], f8, isOutput=False)
    xbf_d = nc.declare_dram_parameter("xbf", [C, L], bf16, isOutput=False)
    wqT_d = nc.declare_dram_parameter("wqT", [P, 2, CT // 2, C], f8, isOutput=False)
    wkT_d = nc.declare_dram_parameter("wkT", [P, 2, CT // 2, C], f8, isOutput=False)
    wvT_d = nc.declare_dram_parameter("wvT", [P, 2, CT // 2, C], f8, isOutput=False)
    woT_d = nc.declare_dram_parameter("woT", [P, 2, CT // 2, C], f8, isOutput=False)
    cp_d = nc.declare_dram_parameter("cparams", [P, CT * 5], f32, isOutput=False)
    gmat_d = nc.declare_dram_parameter("gmat", [P, 2, CT * NG], f8, isOutput=False)
    bmat_d = nc.declare_dram_parameter("bmat", [NG, CT * P], f32, isOutput=False)
    out_d = nc.declare_dram_parameter("out", [C, L], f32, isOutput=True)

    ISUP_ = 1024
    NIB = ISUP_ // P  # 8 i-blocks per superblock
    XCH = 2  # GN processing chunks per x tile ([2, 512] position-pair slices)
    VW = 544  # v tile padded width (512 ch + 2 ones cols, 16B-aligned pair stride)

    with tile.TileContext(nc) as tc:
        with (
            tc.tile_pool(name="consts", bufs=1) as consts,
            tc.tile_pool(name="xt", bufs=4) as xt_pool,
            tc.tile_pool(name="sq", bufs=4) as sq_pool,
            tc.tile_pool(name="ha", bufs=4) as ha_pool,
            tc.tile_pool(name="qk", bufs=2) as qk_pool,
            tc.tile_pool(name="vt", bufs=8) as vt_pool,
            tc.tile_pool(name="pt", bufs=17) as pt_pool,
            tc.tile_pool(name="w", bufs=1) as w_pool,
            tc.tile_pool(name="at", bufs=5) as at_pool,
            tc.tile_pool(name="ot", bufs=5) as ot_pool,
            tc.tile_pool(name="gn", bufs=4) as gn_pool,
            tc.tile_pool(name="ps", bufs=3, space="PSUM") as ps_pool,
            tc.tile_pool(name="paa", bufs=2, space="PSUM") as paa_pool,
            tc.tile_pool(name="pab", bufs=1, space="PSUM") as pab_pool,
            tc.tile_pool(name="pr", bufs=2, space="PSUM") as pr_pool,
        ):
            # ---- constants ----
            epst = consts.tile([P, 1], f32, name="epst")
            nc.vector.memset(epst, float(EPS))
            sh_m2 = consts.tile([P, 1], f32, name="sh_m2")
            nc.vector.memset(sh_m2, -2.0)
            ident = consts.tile([P, P], bf16, name="ident")
            nc.gpsimd.memset(ident, 0.0)
            nc.gpsimd.affine_select(
                out=ident, in_=ident, compare_op=OP.not_equal, fill=1.0,
                base=0, pattern=[[-1, P]], channel_multiplier=1,
            )
            dummy = consts.tile([P, 512], bf16, name="dummy")
            nc.gpsimd.memset(dummy, 0.001)
            ones4 = consts.tile([P, 4], bf16, name="ones4")
            nc.gpsimd.memset(ones4, 1.0)

            # ACT table preloads: only Sqrt + Exp are ever used on ACT (each
            # first use costs a 1.3us table load; keep them off the critical
            # path and avoid any third function that would evict them)
            tblscr = consts.tile([P, 1], f32, name="tblscr")
            nc.scalar.activation(out=tblscr, in_=epst, func=AF.Sqrt, bias=epst, scale=1.0)
            nc.scalar.activation(out=tblscr, in_=epst, func=AF.Exp)

            # ---- DMA: small params first (different queue), then x tiles ----
            gmat = consts.tile([P, 2, CT * NG], f8, name="gmat")
            nc.scalar.dma_start(out=gmat, in_=gmat_d[:, :])
            bmat = consts.tile([NG, CT * P], f32, name="bmat")
            nc.scalar.dma_start(out=bmat, in_=bmat_d[:, :])
            cpt = consts.tile([P, CT * 5], f32, name="cpt")
            nc.scalar.dma_start(out=cpt, in_=cp_d[:, :])
            bq_t = [cpt[:, t * 5 + 0 : t * 5 + 1] for t in range(CT)]
            bk_t = [cpt[:, t * 5 + 1 : t * 5 + 2] for t in range(CT)]
            bo_t = [cpt[:, t * 5 + 2 : t * 5 + 3] for t in range(CT)]
            gam_t = [cpt[:, t * 5 + 3 : t * 5 + 4] for t in range(CT)]
            bet_t = [cpt[:, t * 5 + 4 : t * 5 + 5] for t in range(CT)]

            # PE pre-warm: get the HAM activity window going before x lands
            def warm(n):
                wps = ps_pool.tile([P, 512], f32, name="warm", tag="s")
                for _ in range(n):
                    nc.tensor.matmul(wps, dummy[:, 0:128], dummy, start=True, stop=True)

            warm(4)

            # x arrives as two streams: a 1MB fp8 copy feeding the GN
            # stats + h (halves the critical-path DMA), and the bf16 copy
            # for the residual, which is only needed ~70us later
            x_t = []
            for t in range(CT):
                xt = xt_pool.tile([P, 2, L // 2], f8, name=f"x{t}", tag="x")
                nc.sync.dma_start(out=xt, in_=xf8_d[t * P : (t + 1) * P, :])
                x_t.append(xt)
            xr_t = []
            for t in range(CT):
                xr = xt_pool.tile([P, L], bf16, name=f"xr{t}", tag="xr")
                xr_t.append(xr)

            # weights gated behind x so they don't steal HBM bandwidth from
            # the stats critical path.  The gate is a real data dependency: a
            # corner of each weight tile is written from x_t[3] first, giving
            # the DMA a WAW dependency the scheduler cannot reorder around.
            wq_all = w_pool.tile([P, 2, CT // 2, C], f8, name="wq_all", tag="wq")
            wk_all = w_pool.tile([P, 2, CT // 2, C], f8, name="wk_all", tag="wk")
            wv_all = w_pool.tile([P, 2, CT // 2, C], f8, name="wv_all", tag="wv")
            wo_all = w_pool.tile([P, 2, CT // 2, C], f8, name="wo_all", tag="wo2")
            for d_, wall in ((wqT_d, wq_all), (wkT_d, wk_all), (wvT_d, wv_all), (woT_d, wo_all)):
                nc.scalar.copy(wall[0:1, 0:1, 0:1, 0:1], x_t[CT - 1][0:1, 0:1, 0:1])
                nc.gpsimd.dma_start(out=wall, in_=d_[:, :, :, :])
            for t in range(CT):
                nc.sync.dma_start(out=xr_t[t], in_=xbf_d[t * P : (t + 1) * P, :])

            def w_slice(wall, cp, co):
                return wall[:, :, cp, co * P : (co + 1) * P]

            def w_rhs(wall, cp):
                return wall[:, :, cp, :]

            # ---- GroupNorm stats: PE group-sums tracking the x DMA ----
            # groups (16ch) nest inside 128ch tiles, so stats for tile-pair
            # (0,1) and (2,3) are independent: two accumulator sets, with the
            # first pair finalized while the second still accumulates
            sums_pr = [
                ps_pool.tile([NG, 512], f32, name="gnsum01", tag="s"),
                paa_pool.tile([NG, 512], f32, name="gnsum23", tag="paa"),
            ]
            sqs_pr = [
                ps_pool.tile([NG, 512], f32, name="gnsq01", tag="s"),
                paa_pool.tile([NG, 512], f32, name="gnsq23", tag="paa"),
            ]
            # no ACT squares: ACT's table cache must stay {Sqrt, Exp};
            # GPSIMD is ~3x slower than DVE so it only gets 2 chunks
            def sq_engine(t, ch):
                return nc.gpsimd if (t, ch) == (0, 1) else nc.vector
            sq_tiles = []
            chunks = [(t, ch) for t in range(CT) for ch in range(XCH)]

            def sum_mm(k):
                t, ch = chunks[k]
                nc.tensor.matmul(
                    sums_pr[t // 2],
                    gmat[:, :, t * NG : (t + 1) * NG],
                    x_t[t][:, :, ch * 512 : (ch + 1) * 512],
                    start=(k % (2 * XCH) == 0),
                    stop=(k % (2 * XCH) == 2 * XCH - 1),
                    perf_mode=mybir.MatmulPerfMode.DoubleRow,
                )

            def sq_make(k):
                t, ch = chunks[k]
                sq = sq_pool.tile([P, 2, 512], f8, name=f"sq{k}", tag="sq")
                xs = x_t[t][:, :, ch * 512 : (ch + 1) * 512]
                sq_engine(t, ch).tensor_mul(sq, xs, xs)
                sq_tiles.append(sq)

            def sq_mm(k):
                t = chunks[k][0]
                nc.tensor.matmul(
                    sqs_pr[t // 2],
                    gmat[:, :, t * NG : (t + 1) * NG],
                    sq_tiles[k],
                    start=(k % (2 * XCH) == 0),
                    stop=(k % (2 * XCH) == 2 * XCH - 1),
                    perf_mode=mybir.MatmulPerfMode.DoubleRow,
                )

            # finalize chain for one tile-pair: issued mid-loop so the DVE
            # ops sit BEFORE the second pair's square chunks in the FIFO
            stat2_pr = []

            def finalize_pair(pr):
                red = gn_pool.tile([NG, 2], f32, name=f"red{pr}", tag="red")
                nc.vector.tensor_reduce(
                    out=red[:, 0:1], in_=sums_pr[pr], axis=mybir.AxisListType.X, op=OP.add
                )
                nc.vector.tensor_reduce(
                    out=red[:, 1:2], in_=sqs_pr[pr], axis=mybir.AxisListType.X, op=OP.add
                )
                gss = gn_pool.tile([NG, 2], f32, name=f"gss{pr}", tag="gss")
                nc.vector.tensor_scalar_mul(gss, red, float(1.0 / (GS * L)))
                nvar = gn_pool.tile([NG, 1], f32, name=f"nvar{pr}", tag="nv")
                nc.vector.scalar_tensor_tensor(
                    out=nvar, in0=gss[:, 0:1], scalar=gss[:, 0:1],
                    in1=gss[:, 1:2], op0=OP.mult, op1=OP.subtract,
                )
                stat2 = gn_pool.tile([NG, 2], f32, name=f"stat2{pr}", tag="st2")
                nc.scalar.activation(
                    out=stat2[:, 0:1], in_=nvar, func=AF.Sqrt, bias=epst[0:NG, :], scale=-1.0
                )
                nc.vector.reciprocal(out=stat2[:, 0:1], in_=stat2[:, 0:1])
                # stat2[:,1] = -mu * rstd
                nc.vector.tensor_scalar(
                    out=stat2[:, 1:2], in0=gss[:, 0:1], scalar1=stat2[:, 0:1],
                    scalar2=-1.0, op0=OP.mult, op1=OP.mult,
                )
                stat2_pr.append(stat2)

            # lag the sq matmul one chunk behind the sum matmul so the PE
            # always has ready work
            for k in range(len(chunks)):
                sq_make(k)
                sum_mm(k)
                if k > 0:
                    sq_mm(k - 1)
                if k == 2 * XCH:
                    finalize_pair(0)
            sq_mm(len(chunks) - 1)
            finalize_pair(1)

            # ---- per-channel scale/shift -> h (engines: t0 ACT, t1 GPSIMD,
            # t2 DVE, t3 split ACT+DVE; gamma is folded into bmat on host) ----
            h_t = []
            for cp in range(CT // 2):
                hp = ha_pool.tile([P, 2, L], f8, name=f"h{cp}", tag="ha")
                h_t.append(hp)
            h_sched = {
                (0, 0): nc.scalar, (0, 1): nc.scalar,
                (1, 0): nc.gpsimd, (1, 1): nc.gpsimd,
                (2, 0): nc.vector, (2, 1): nc.vector,
                (3, 0): nc.scalar, (3, 1): nc.vector,
            }
            for t in range(CT):
                if t == 2:
                    # PE fillers keep HAM warm while finalize23 + h run
                    warm(8)
                bps = ps_pool.tile([P, 2], f32, name=f"bps{t}", tag="s")
                nc.tensor.matmul(
                    bps, bmat[:, t * P : (t + 1) * P], stat2_pr[t // 2],
                    start=True, stop=True,
                )
                sc = gn_pool.tile([P, 1], f32, name=f"sc{t}", tag="sc")
                nc.vector.tensor_copy(sc, bps[:, 0:1])
                bc = gn_pool.tile([P, 1], f32, name=f"bc{t}", tag="bc")
                nc.vector.tensor_add(bc, bps[:, 1:2], bet_t[t])
                for hh in range(2):
                    hsl = h_t[t // 2][:, t % 2, hh * 1024 : (hh + 1) * 1024]
                    xsl = x_t[t][:, hh, :]
                    eng = h_sched[(t, hh)]
                    if eng is nc.scalar:
                        nc.scalar.activation(
                            out=hsl, in_=xsl, func=AF.Identity, bias=bc, scale=sc
                        )
                    else:
                        eng.tensor_scalar(
                            out=hsl, in0=xsl, scalar1=sc, scalar2=bc,
                            op0=OP.mult, op1=OP.add,
                        )
            warm(10)

            # ---- Q, K projections: [co, l], paired fp8 for DoubleRow S^T ----
            q_t, k_t = [], []
            for cp in range(CT // 2):
                qt = qk_pool.tile([P, 2, L], f8, name=f"q{cp}", tag="q")
                kt = qk_pool.tile([P, 2, L], f8, name=f"k{cp}", tag="k")
                q_t.append(qt)
                k_t.append(kt)
            qk_blk = 0
            for wts, dst, bias in ((wq_all, q_t, bq_t), (wk_all, k_t, bk_t)):
                for co in range(CT):
                    for lg in range(2):
                        psh = [
                            ps_pool.tile([P, 512], f32, name=f"pq{co}_{lg}_{ih}", tag="s")
                            for ih in range(2)
                        ]
                        for cp in range(CT // 2):
                            for ih in range(2):
                                nc.tensor.matmul(
                                    psh[ih],
                                    w_slice(wts, cp, co),
                                    h_t[cp][:, :, lg * 1024 + ih * 512 : lg * 1024 + (ih + 1) * 512],
                                    start=(cp == 0),
                                    stop=(cp == CT // 2 - 1),
                                    perf_mode=mybir.MatmulPerfMode.DoubleRow,
                                )
                        for ih in range(2):
                            od = dst[co // 2][
                                :, co % 2, lg * 1024 + ih * 512 : lg * 1024 + (ih + 1) * 512
                            ]
                            if (qk_blk + ih) % 2 == 0:
                                nc.scalar.activation(
                                    out=od, in_=psh[ih], func=AF.Identity, bias=bias[co], scale=1.0
                                )
                            else:
                                nc.vector.tensor_scalar_add(out=od, in0=psh[ih], scalar1=bias[co])
                        qk_blk += 1

            # ---- V^T projection: [l, co+ones], paired fp8 for DoubleRow AV ----
            v_t = []
            for jp in range(LT // 2):
                vt = vt_pool.tile([P, 2, VW], f8, name=f"v{jp}", tag="v")
                for pi in range(2):
                    nc.scalar.activation(
                        out=vt[:, pi, 512:514], in_=ones4[:, 0:2], func=AF.Identity, scale=1.0
                    )
                v_t.append(vt)

            def v_proj(lt):
                ps = paa_pool.tile([P, 512], f32, name=f"pv{lt}", tag="paa")
                for cp in range(CT // 2):
                    nc.tensor.matmul(
                        ps,
                        h_t[cp][:, :, lt * P : (lt + 1) * P],
                        w_rhs(wv_all, cp),
                        start=(cp == 0),
                        stop=(cp == CT // 2 - 1),
                        perf_mode=mybir.MatmulPerfMode.DoubleRow,
                    )
                nc.vector.tensor_copy(v_t[lt // 2][:, lt % 2, 0:512], ps)

            # ---- attention + interleaved output projection ----
            a_t = []
            for cp in range(CT // 2):
                at = ha_pool.tile([P, 2, L], f8, name=f"a{cp}", tag="ha")
                a_t.append(at)

            def st_setup(sup):
                pts = []
                for jp in range(LT // 2):
                    ptp = pt_pool.tile([P, 2, ISUP_], f8, name=f"pt{sup}_{jp}", tag="pt")
                    pts.append(ptp)
                return pts

            def st_j(sup, pts, j):
                i0 = sup * ISUP_
                psh = [
                    ps_pool.tile([P, 512], f32, name=f"pst{sup}_{j}_{ih}", tag="s")
                    for ih in range(2)
                ]
                for cp in range(CT // 2):
                    for ih in range(2):
                        nc.tensor.matmul(
                            psh[ih],
                            k_t[cp][:, :, j * P : (j + 1) * P],
                            q_t[cp][:, :, i0 + ih * 512 : i0 + (ih + 1) * 512],
                            start=(cp == 0),
                            stop=(cp == CT // 2 - 1),
                            perf_mode=mybir.MatmulPerfMode.DoubleRow,
                        )
                # exp(scale*s - 2): shift keeps fp8 range safe, cancels in
                # the normalization
                for ih in range(2):
                    nc.scalar.activation(
                        out=pts[j // 2][:, j % 2, ih * 512 : (ih + 1) * 512],
                        in_=psh[ih], func=AF.Exp, scale=SCALE, bias=sh_m2,
                    )

            def transpose_out(sup, ib, at):
                iblk = sup * ISUP_ + ib * P
                for cc in range(CT):
                    ptr = pr_pool.tile([P, P], bf16, name=f"ptr{sup}_{ib}_{cc}", tag="ptr")
                    nc.tensor.transpose(ptr, at[:, cc * P : (cc + 1) * P], ident)
                    if cc % 2 == 0:
                        nc.vector.tensor_copy(
                            a_t[cc // 2][:, cc % 2, iblk : iblk + P], ptr
                        )
                    else:
                        nc.scalar.activation(
                            out=a_t[cc // 2][:, cc % 2, iblk : iblk + P],
                            in_=ptr, func=AF.Identity, scale=1.0,
                        )

            def av_phase(sup, pts, next_pts=None, o_hooks=None):
                prev = None
                for ib in range(NIB):
                    if o_hooks is not None and ib in o_hooks:
                        o_hooks[ib]()
                    if next_pts is not None:
                        st_j(sup + 1, next_pts, 2 * ib)
                        st_j(sup + 1, next_pts, 2 * ib + 1)
                    pa_a = paa_pool.tile([P, 256], f32, name=f"paa{sup}_{ib}", tag="paa")
                    pa_b = pab_pool.tile([P, 258], f32, name=f"pab{sup}_{ib}", tag="pab")
                    for jp in range(LT // 2):
                        lhs = pts[jp][:, :, ib * P : (ib + 1) * P]
                        nc.tensor.matmul(
                            pa_a, lhs, v_t[jp][:, :, 0:256],
                            start=(jp == 0), stop=(jp == LT // 2 - 1),
                            perf_mode=mybir.MatmulPerfMode.DoubleRow,
                        )
                        nc.tensor.matmul(
                            pa_b, lhs, v_t[jp][:, :, 256:514],
                            start=(jp == 0), stop=(jp == LT // 2 - 1),
                            perf_mode=mybir.MatmulPerfMode.DoubleRow,
                        )
                    rec = gn_pool.tile([P, 1], f32, name=f"rec{sup}_{ib}", tag="rec")
                    nc.vector.reciprocal(out=rec, in_=pa_b[:, 256:257])
                    at = at_pool.tile([P, 512], bf16, name=f"aT{sup}_{ib}", tag="aT")
                    if ib % 2 == 0:
                        nc.scalar.activation(
                            out=at[:, 0:256], in_=pa_a, func=AF.Identity, scale=rec
                        )
                        nc.vector.tensor_scalar_mul(at[:, 256:512], pa_b[:, 0:256], rec)
                    else:
                        nc.vector.tensor_scalar_mul(at[:, 0:256], pa_a, rec)
                        nc.scalar.activation(
                            out=at[:, 256:512], in_=pa_b[:, 0:256], func=AF.Identity, scale=rec
                        )
                    if prev is not None:
                        transpose_out(sup, ib - 1, prev)
                    prev = at
                transpose_out(sup, NIB - 1, prev)

            def o_block(l0, width, tag):
                pool = ps_pool if tag == "s" else paa_pool
                for co in range(CT):
                    ps = pool.tile([P, width], f32, name=f"po{l0}_{co}", tag=tag)
                    for cp in range(CT // 2):
                        nc.tensor.matmul(
                            ps,
                            w_slice(wo_all, cp, co),
                            a_t[cp][:, :, l0 : l0 + width],
                            start=(cp == 0),
                            stop=(cp == CT // 2 - 1),
                            perf_mode=mybir.MatmulPerfMode.DoubleRow,
                        )
                    ot = ot_pool.tile([P, width], f32, name=f"o{l0}_{co}", tag="o")
                    nc.vector.scalar_tensor_tensor(
                        out=ot, in0=ps, scalar=bo_t[co],
                        in1=xr_t[co][:, l0 : l0 + width],
                        op0=OP.add, op1=OP.add,
                    )
                    nc.sync.dma_start(
                        out=out_d[co * P : (co + 1) * P, l0 : l0 + width],
                        in_=ot,
                    )

            # S^T superblock 0 with the V projection interleaved (V is
            # independent of S; it fills the PE while ACT drains the exps)
            pts0 = st_setup(0)
            for j in range(LT):
                st_j(0, pts0, j)
                v_proj(j)

            # software pipeline: O-proj of sup0 runs while sup1's S^T keeps
            # the PE busy; sup1's O runs as quarter-blocks interleaved into
            # the final AV loop so the serial tail is one 256-wide block
            pts1 = st_setup(1)
            av_phase(0, pts0, next_pts=pts1)
            o_block(0, 512, "paa")
            o_block(512, 512, "paa")
            o_hooks = {
                3: lambda: o_block(1024, 256, "s"),
                5: lambda: o_block(1280, 256, "s"),
                7: lambda: o_block(1536, 256, "s"),
            }
            av_phase(1, pts1, o_hooks=o_hooks)
            o_block(1792, 256, "s")

    nc.compile()
    return nc


def _pair_pack(WT):
    """[C_in, C_out] -> [P, 2, CT//2, C_out] fp8, pairing ci-chunks (2cp, 2cp+1)."""
    w4 = WT.reshape(CT // 2, 2, P, C).transpose(2, 1, 0, 3)
    return np.ascontiguousarray(w4).astype(ml_dtypes.float8_e4m3)


def _prep_maps(inputs):
    x = np.asarray(inputs["x"], dtype=np.float32)
    Wq = np.asarray(inputs["Wq"], dtype=np.float32)
    Wk = np.asarray(inputs["Wk"], dtype=np.float32)
    Wv = np.asarray(inputs["Wv"], dtype=np.float32)
    Wo = np.asarray(inputs["Wo"], dtype=np.float32)
    bq = np.asarray(inputs["bq"], dtype=np.float32)
    bk = np.asarray(inputs["bk"], dtype=np.float32)
    bv = np.asarray(inputs["bv"], dtype=np.float32)
    bo = np.asarray(inputs["bo"], dtype=np.float32)
    gam = np.asarray(inputs["gn_gamma"], dtype=np.float32)
    bet = np.asarray(inputs["gn_beta"], dtype=np.float32)

    bo_eff = bo + Wo @ bv  # v-bias commutes through attention weights (rows sum to 1)

    cp_ctile = np.stack([bq, bk, bo_eff.astype(np.float32), gam, bet], axis=1)  # [C, 5]
    cparams = cp_ctile.reshape(CT, P, 5).transpose(1, 0, 2).reshape(P, CT * 5)

    # one-hot group matrices: gmat[p, t*NG+g] = 1 iff group(t*P+p) == g
    # (PE group-sum lhsT); bmat[g, t*P+p] same predicate (stats broadcast lhsT)
    gmat = np.zeros((P, CT * NG), dtype=np.float32)  # replicated below
    bmat = np.zeros((NG, CT * P), dtype=np.float32)
    for t in range(CT):
        for p in range(P):
            c = t * P + p
            g = c // GS
            gmat[p, t * NG + g] = 1.0
            # gamma folded in: bps from bmat gives (rstd*gamma, -mu*rstd*gamma)
            bmat[g, c] = gam[c]

    shared = {
        "wqT": _pair_pack(Wq.T),
        "wkT": _pair_pack(Wk.T),
        "wvT": _pair_pack(Wv.T),
        "woT": _pair_pack(Wo.T),
        "cparams": np.ascontiguousarray(cparams, dtype=np.float32),
        "gmat": np.ascontiguousarray(
            np.broadcast_to(gmat[:, None, :], (P, 2, CT * NG))
        ).astype(ml_dtypes.float8_e4m3),
        "bmat": np.ascontiguousarray(bmat, dtype=np.float32),
    }
    in_maps = []
    for i in range(B):
        m = dict(shared)
        xi = np.ascontiguousarray(x[i])
        m["xf8"] = xi.astype(ml_dtypes.float8_e4m3)
        m["xbf"] = xi.astype(ml_dtypes.bfloat16)
        in_maps.append(m)
    return in_maps


def _install_trace_hook():
    """The image's antenv lacks axon_hooks; recreate the shim so bass_utils
    can reach the NTFF profiler in libaxon_pjrt.so (for exec_time_ns)."""
    import types

    if "antenv.axon_hooks" in sys.modules:
        return True
    try:
        from trn_agent_boot.trn_boot import _ntff_profile_via_ctypes

        hook = _ntff_profile_via_ctypes("/opt/axon/libaxon_pjrt.so")
        if hook is None:
            return False
        mod = types.ModuleType("antenv.axon_hooks")
        mod._hook = hook
        mod.get_axon_ntff_profile_hook = lambda: mod._hook
        mod.set_axon_ntff_profile_hook = lambda h: setattr(mod, "_hook", h)
        sys.modules["antenv.axon_hooks"] = mod
        return True
    except Exception as e:  # pragma: no cover
        print(f"trace hook install failed: {e}", file=sys.stderr)
        return False


def kernel(**inputs):
    global LAST_RESULT
    from concourse import bass_utils
    from concourse.bass_utils import run_bass_kernel_spmd

    trace = os.environ.get("KERNEL_TRACE", "0") == "1"
    if trace:
        trace = _install_trace_hook()
        # skip the remote-bucket artifact upload; keep everything local
        bass_utils.upload_artifacts = lambda tmpdir: f"local://{tmpdir}"
    in_maps = _prep_maps(inputs)
    nc = _build_nc()
    res = run_bass_kernel_spmd(nc, in_maps, core_ids=list(range(B)), trace=trace)
    LAST_RESULT = res
    out = np.stack([np.asarray(res.results[i]["out"]) for i in range(B)], axis=0)
    return out.astype(np.float32)
